# revision 32
# baseline (speedup 1.0000x reference)
"""Trainium2 Bass kernel: per-voxel eigenvalues of 3x3 symmetric matrices.

Input  X: (2, 9, 96, 96, 96) float32 -- each voxel holds a row-major 3x3
matrix in the channel dim.  Output: (2, 3, 96, 96, 96) float32, the
eigenvalues of the symmetrized matrix, ascending in the channel dim.

Strategy: embarrassingly parallel over voxels.  The 884736 voxels per batch
are sharded 8 ways (110592 = 128 partitions x 864 free per core per batch).
Each core runs a straight-line closed-form trigonometric eigensolver:

    q   = tr/3,  aq/bq/cq = diag - q,  D/E/F = doubled off-diagonals
    p2q = p2x/4 = (cq^2 - aq*bq) + (D^2+E^2+F^2)/4   [deviatoric identity:
          aq+bq+cq = 0  =>  aq^2+bq^2+cq^2 = 2(cq^2 - aq*bq)]
    det = det(A - qI) = aq*bq*cq + (D/2)*(E/2)*F - (aq*F^2+bq*E^2+cq*D^2)/4
    r   = det / (2 p^3),  p = sqrt(p2x/12)   [/4 folded into exp biases]
    asin(r)/2 = atan(r * sigmoid(-0.5*ln(1-r^2)))
    lmax = q + 2p sin(-asin(r)/3 + 2pi/3), lmin = q + 2p sin(asin(r)/3-pi/3)
    lmid = tr - lmax - lmin        (eigenvalue-sum identity; saves one Sin)

Precision: inputs are downcast to bf16 on the host (half the HBM traffic,
2x DVE rate); the linear/quadratic front-end runs in bf16, the
cancellation-prone det/p2 combines and the ln/exp/atan chain run in fp32,
outputs are stored bf16 and upcast on the host.  Measured rel err ~5.7e-3
(gate 2e-2).

Scheduling: two 864-wide chunks per rep, 4 slot parities for deep
cross-chunk overlap, loads software-pipelined one step ahead, an ACT
table preload at t=0, and per-chunk ACT phase grouping (natural_log_exp
-> sigmoid -> trig) so a rep pays only 3 table loads.  Work is split
DVE (bf16 2x tensor ops) / GPSIMD (mixed-dtype combines) / Scalar
(transcendental chain), emission-ordered so the p2 path reaches the
scalar engine early.
"""

import sys

if "/opt/trn_rl_repo" not in sys.path:
    sys.path.insert(0, "/opt/trn_rl_repo")

import math

import numpy as np

N_CORES = 8
B = 2
DHW = 96 * 96 * 96          # 884736 voxels per batch
PER = DHW // N_CORES        # 110592 voxels per batch per core
P = 128                     # SBUF partitions
FB = PER // P               # 864 free elems per batch per core
FT = B * FB                 # packed free dim per core (both batches)
CHUNKS = [864, 864]
NCHUNK = len(CHUNKS)
TP = max(CHUNKS)

#  p2q = p2x/4 is what reaches Ln; the /4 is folded into the exp biases.
E1_BIAS = (0.5 * math.log(54.0) + 1.5 * math.log(2.0)
           - 1.5 * math.log(4.0))                       # ln(1/(2 p^3)) offset
P2_BIAS = -0.5 * math.log(3.0) + 0.5 * math.log(4.0)    # ln(2p) offset
LN_EPS = 1e-20
TWO_PI_3 = 2.0 * math.pi / 3.0
PI_3 = math.pi / 3.0
R2_CLAMP = 1.0 - 2.0 ** -23

_CACHE = {}


def _build(split_waits=True, nrep=1):
    import concourse.bass as bass
    import concourse.tile as tile
    from concourse import mybir

    fp32 = mybir.dt.float32
    bf16 = mybir.dt.bfloat16
    AF = mybir.ActivationFunctionType
    OP = mybir.AluOpType

    nc = bass.Bass("TRN2", target_bir_lowering=False, debug=False,
                   num_devices=N_CORES)
    x = nc.dram_tensor("x", [9, P, FT], bf16, kind="ExternalInput").ap()
    y = nc.dram_tensor("y", [3, P, FT], bf16, kind="ExternalOutput").ap()

    # Activation biases must exist as SBUF const APs before use.
    for cval in (E1_BIAS, P2_BIAS, LN_EPS, TWO_PI_3, PI_3, -PI_3, 1.0):
        cval = float(cval)
        if (fp32, cval) not in nc.const_aps.aps:
            ctens = nc.alloc_sbuf_tensor(f"const-f32-{cval}", [128, 1], fp32)
            nc.gpsimd.memset(ctens.ap(), cval)
            nc.const_aps.aps[(fp32, cval)] = ctens.ap()
    nc.all_engine_barrier()

    V, G, S = nc.vector, nc.gpsimd, nc.scalar

    with tile.TileContext(nc) as tc:
        with tc.tile_pool(name="sl", bufs=1) as pool:
            n_slots = {"h": 16, "f": 7}
            n_par = 4
            free_slots = {
                par: {k: [par * n + s for s in range(n)]
                      for k, n in n_slots.items()}
                for par in range(n_par)
            }
            name2slot = {}
            tiles = {}

            cur_tp = [TP]
            cur_key = [0]
            cur_par = [0]

            def alloc(name, dt):
                k = "h" if dt == bf16 else "f"
                par = cur_par[0]
                s = free_slots[par][k].pop(0)
                name2slot[name] = (par, k, s)
                t = pool.tile([P, cur_tp[0]], dt, tag=f"s{s}-{k}")
                tiles[name] = t
                return t

            def rel(*names):
                for name in names:
                    par, k, s = name2slot.pop(name)
                    free_slots[par][k].append(s)
                    del tiles[name]

            def tt(eng, dst, a, b, op, dt=bf16):
                d = alloc(dst, dt)
                fn = {"add": eng.tensor_add, "sub": eng.tensor_sub,
                      "mul": eng.tensor_mul}[op]
                fn(d[:, :], tiles[a][:, :], tiles[b][:, :])
                return d

            def ts(eng, dst, a, c, dt=bf16):
                d = alloc(dst, dt)
                eng.tensor_scalar_mul(d[:, :], tiles[a][:, :], float(c))
                return d

            act_insts = {}

            def act(dst, src, func, scale=1.0, bias=0.0, dt=fp32):
                d = alloc(dst, dt)
                inst = S.activation(d[:, :], tiles[src][:, :], func,
                                    bias=float(bias), scale=float(scale))
                act_insts[(cur_key[0], dst)] = inst
                return d

            # Preload the ln/exp ACT table set at t=0 (dummy 1-elem Ln) so
            # the first real activation doesn't pay the ~2.3us table load on
            # the critical path -- it overlaps the first chunk's DMA+DVE.
            warm = pool.tile([P, 1], fp32, tag="actwarm", name="actwarm")
            S.activation(warm[:, :], nc.const_aps.aps[(fp32, 1.0)],
                         AF.Ln, bias=0.0, scale=1.0)

            steps = [(rep, ci) for rep in range(nrep) for ci in range(NCHUNK)]
            offs = [sum(CHUNKS[:ci]) for ci in range(NCHUNK)]

            def do_load(i):
                """Issue the 9 channel-plane loads for step i (prefetched one
                step ahead so they sit before step i-1's stores in the SP
                HWDGE FIFO and overlap its compute)."""
                rep, ci = steps[i]
                par = i % n_par
                cur_par[0] = par
                cur_tp[0] = CHUNKS[ci]
                sl2 = slice(offs[ci], offs[ci] + CHUNKS[ci])
                for ch in (1, 3, 0, 4, 8, 2, 6, 5, 7):
                    t = alloc(f"x{ch}@{par}", bf16)
                    nc.sync.dma_start(out=t[:, :], in_=x[ch][:, sl2])

            def do_compute(i):
                rep, ci = steps[i]
                par = i % n_par
                cur_par[0] = par
                cur_key[0] = (rep, ci)
                cur_tp[0] = CHUNKS[ci]
                sl2 = slice(offs[ci], offs[ci] + CHUNKS[ci])

                def xn(ch):
                    return f"x{ch}@{par}"

                # ---- linear stage (bf16).  qn = -tr/3; aq/bq/cq = xk + qn.
                # Emission order prioritizes the p2 path: the ACT ln/exp
                # chain depends on p2q only, so getting it out early lets the
                # scalar engine start while DVE grinds the det products.
                tt(V, "t0", xn(0), xn(4), "add")
                tt(V, "tr", "t0", xn(8), "add")
                rel("t0")
                ts(V, "qn", "tr", -1.0 / 3.0)
                tt(G, "aq", xn(0), "qn", "add")
                tt(G, "bq", xn(4), "qn", "add")
                tt(G, "cq", xn(8), "qn", "add")
                rel(xn(0), xn(4), xn(8))
                tt(V, "D", xn(1), xn(3), "add")
                tt(V, "E", xn(2), xn(6), "add")
                tt(V, "F", xn(5), xn(7), "add")
                rel(xn(1), xn(3), xn(2), xn(6), xn(5), xn(7))

                # ---- p2q = p2x/4 = (cq^2 - aq*bq) + (ddq+eeq+ffq)/4
                tt(V, "ddq", "D", "D", "mul")
                tt(V, "eeq", "E", "E", "mul")
                tt(V, "ffq", "F", "F", "mul")
                tt(V, "m", "aq", "bq", "mul")        # aq*bq
                tt(V, "cc", "cq", "cq", "mul")       # cq^2
                tt(V, "df", "cc", "m", "sub")        # cc - aq*bq
                rel("cc")
                tt(V, "s1", "ddq", "eeq", "add")
                tt(V, "p1", "s1", "ffq", "add")
                rel("s1")
                ts(V, "p1q", "p1", 0.25)
                rel("p1")
                tt(G, "p2q", "df", "p1q", "add", dt=fp32)
                rel("df", "p1q")

                # ---- det(A - qI) = G1 + d*e*F - (aq*ffq+bq*eeq+cq*ddq)/4
                # (d, e are halved off-diagonals: d*e*F = DEF/4)
                ts(V, "d", "D", 0.5)
                ts(V, "e", "E", 0.5)
                rel("D", "E")
                tt(V, "w1", "d", "e", "mul")
                tt(V, "w2", "w1", "F", "mul")
                rel("d", "e", "F", "w1")
                tt(G, "G1", "m", "cq", "mul")        # aq*bq*cq
                tt(V, "v1", "aq", "ffq", "mul")
                tt(V, "v2", "cq", "ddq", "mul")
                tt(V, "v3", "bq", "eeq", "mul")
                rel("aq", "bq", "cq", "m", "ddq", "eeq", "ffq")
                tt(G, "v4", "v1", "v2", "add")
                tt(G, "v5", "v4", "v3", "add")
                rel("v1", "v2", "v3", "v4")
                ts(V, "v5q", "v5", 0.25)
                rel("v5")
                tt(G, "tG", "G1", "w2", "add")
                rel("G1", "w2")
                tt(G, "det", "tG", "v5q", "sub", dt=fp32)
                rel("tG", "v5q")

                # ---- r = det/(2p^3), asin(r) = 2*atan(r/(1+sqrt(1-r^2)))
                act("lnp2", "p2q", AF.Ln, bias=LN_EPS)             # ln(p2x/4)
                rel("p2q")
                act("e1", "lnp2", AF.Exp, scale=-1.5, bias=E1_BIAS)
                act("P2", "lnp2", AF.Exp, scale=0.5, bias=P2_BIAS, dt=bf16)  # 2p
                rel("lnp2")
                tt(V, "rr", "det", "e1", "mul", dt=fp32)           # r
                rel("det", "e1")
                tt(G, "r2", "rr", "rr", "mul", dt=fp32)
                d_ = alloc("r2c", fp32)
                V.tensor_scalar_min(d_[:, :], tiles["r2"][:, :], R2_CLAMP)
                rel("r2")
                act("ln1mr2", "r2c", AF.Ln, scale=-1.0, bias=1.0)  # ln(1-r^2)
                rel("r2c")
                # 1/(1+sqrt(1-r^2)) = sigmoid(-0.5*ln(1-r^2))
                act("inv1s", "ln1mr2", AF.Sigmoid, scale=-0.5)
                rel("ln1mr2")
                tt(G, "t2", "rr", "inv1s", "mul", dt=fp32)         # in [-1, 1]
                rel("rr", "inv1s")
                act("at", "t2", AF.Arctan)                         # asin(r)/2
                rel("t2")
                act("c1", "at", AF.Sin, scale=-2.0 / 3.0, bias=TWO_PI_3,
                    dt=bf16)
                # c2n = -sin(pi/3 - asin(r)/3) so lmin = P2*c2n - qn
                act("c2n", "at", AF.Sin, scale=2.0 / 3.0, bias=-PI_3, dt=bf16)
                rel("at")
                tt(V, "m1", "P2", "c1", "mul")
                tt(V, "lmax", "m1", "qn", "sub")                   # q + P2*c1
                rel("c1", "m1")
                tt(G, "m2", "P2", "c2n", "mul")
                tt(G, "lmin", "m2", "qn", "sub")                   # q - P2*|c2n|
                rel("c2n", "m2", "P2", "qn")
                tt(V, "su", "lmax", "lmin", "add")
                tt(G, "lmid", "tr", "su", "sub")                   # tr - su
                rel("su", "tr")

                # ---- store ascending eigenvalues (bf16), earliest first
                for k, name in ((2, "lmax"), (0, "lmin"), (1, "lmid")):
                    nc.sync.dma_start(out=y[k][:, sl2], in_=tiles[name][:, :])
                rel("lmin", "lmid", "lmax")

            # Software-pipelined issue order: step i+1's loads are emitted
            # before step i's compute+stores, so the SP HWDGE FIFO drains the
            # next step's input planes while this step computes.
            do_load(0)
            for i in range(len(steps)):
                if i + 1 < len(steps):
                    do_load(i + 1)
                do_compute(i)

            # Group ACT ops by table set across the chunks of a rep: phases
            # are lnexp {lnp2,e1,P2,ln1mr2} -> sigmoid {inv1s,at} -> trig
            # {c1,c2n}.  Delay chunk ci's first op of a phase until chunk
            # ci+1's last op of the previous phase -> 3 table loads per rep.
            if NCHUNK > 1:
                from concourse.bass import _add_dep_helper
                for rep in range(nrep):
                    for ci in range(NCHUNK - 1):
                        for first, last in (("inv1s", "ln1mr2"), ("c1", "at")):
                            a = act_insts.get(((rep, ci), first))
                            b = act_insts.get(((rep, ci + 1), last))
                            if a is not None and b is not None:
                                _add_dep_helper(a.ins, b.ins, sync=False,
                                                reason="act-table-grouping")

    if split_waits:
        _split_multi_waits(nc, mybir)
    return nc


def _split_multi_waits(nc, mybir):
    """walrus codegen allows a single sync-wait slot per TPB instruction;
    hoist extra waits onto standalone NoOps on the same engine."""
    for f in nc.m.functions:
        for blk in f.blocks:
            il = blk.instructions
            i = 0
            while i < len(il):
                inst = il[i]
                si = inst.sync_info
                if si is not None and si.on_wait and len(si.on_wait) > 1:
                    waits = list(si.on_wait)
                    for w in waits[:-1]:
                        nop = mybir.InstNoOp(
                            name=nc.get_next_instruction_name(),
                            engine=inst.engine,
                            ins=[],
                            outs=[],
                            sync_info=mybir.SyncInfo(on_wait=[w], on_update=[]),
                            bass_nofuse=True,
                        )
                        il.insert(i, nop)
                        i += 1
                    si.on_wait = waits[-1:]
                i += 1


def get_program():
    if "nc" not in _CACHE:
        _CACHE["nc"] = _build()
    return _CACHE["nc"]


def _bf16():
    import ml_dtypes
    return ml_dtypes.bfloat16


def shard_inputs(X):
    """X: (2,9,96,96,96) f32 -> list of per-core {"x": (9,128,1728) bf16}."""
    x = np.asarray(X, dtype=np.float32).reshape(B, 9, DHW)
    bf16 = _bf16()
    maps = []
    for c in range(N_CORES):
        # (B, 9, PER) -> (9, P, B, FB) -> (9, P, FT)
        slab = x[:, :, c * PER:(c + 1) * PER].reshape(B, 9, P, FB)
        xc = np.ascontiguousarray(slab.transpose(1, 2, 0, 3)).reshape(9, P, FT)
        maps.append({"x": xc.astype(bf16)})
    return maps


def unshard_outputs(results):
    out = np.empty((B, 3, DHW), dtype=np.float32)
    for c, r in enumerate(results):
        yc = np.asarray(r["y"]).astype(np.float32)
        yc = yc.reshape(3, P, B, FB).transpose(2, 0, 1, 3)
        out[:, :, c * PER:(c + 1) * PER] = yc.reshape(B, 3, PER)
    return out.reshape(B, 3, 96, 96, 96)


def kernel(X):
    from concourse.bass_utils import run_bass_kernel_spmd

    nc = get_program()
    in_maps = shard_inputs(np.asarray(X))
    res = run_bass_kernel_spmd(nc, in_maps, list(range(N_CORES)))
    return unshard_outputs(res.results)
